# revision 24
# baseline (speedup 1.0000x reference)
"""DualAttention (DANet position+channel attention) on 8 TRN2 NeuronCores.

Sharding: core c handles sample b=c//2, query-half h=c%2 (2048 of 4096
spatial positions). Each core gets its sample's full xf=[512,4096] with its
own half's columns FIRST (attention sums over keys/positions are
permutation-invariant, so column order only matters for which queries the
core computes). BN batch stats are combined with a 4KB AllReduce over all
8 cores.

Per-core device pipeline (fp32r matmuls unless noted; PE kept dense to
avoid HAM re-throttle):
  B: q,k = wq/wk @ x                  -> bf16 (padded to 128 partitions)
  D1: ce = x x^T via PE-transposed x chunks (fp32r)
  D2: channel softmax rows: exp(rowmin - ce)/sum (stable form of
      softmax(rowmax - ce)); gamma_ca and 1/rowsum folded in
  C:  vT[m,c] = x.T chunks @ wvT -> bf16 (PE busy during D2's ACT/DVE work)
  D3: cattn^T via PE transpose
  E (per 512-query group, software-pipelined):
      ca = cattn^T.T @ x;  E^T[m-chunk,n] = k_chunk^T q_g (bf16);
      exp on ACT (no max subtraction: |energy| <= ~11 for this data);
      PV accumulates pa over 32 m-chunks; rowsum via all-ones lhsT;
      epilogue s = gpa*pa/rowsum + ca + 2x on ACT+DVE;
      fuse y(g) = wfT.T @ s(g) is emitted DURING group g+1 so PE never
      stalls on the epilogue; y spilled to DRAM + sum/sumsq stats
  F: AllReduce [128,8] stats; a=scale*rsqrt(var+eps), b=bias-mean*a;
     y reloads prefetched under the collective; relu(y*a+b) -> output
"""
import numpy as np

B, C, H, W = 4, 512, 64, 64
N = H * W            # 4096
C8 = C // 8          # 64
NCORES = 8
NH = N // 2          # 2048 queries per core
P = 128
NB = 512             # free-dim block
BN_EPS = 1e-5
CNT = float(B * H * W)  # BN count per channel = 16384

_CACHE = {}


def _build_program():
    import concourse.tile as tile
    from concourse import bacc, mybir
    f32 = mybir.dt.float32
    f32r = mybir.dt.float32r
    bf16 = mybir.dt.bfloat16
    AX = mybir.AxisListType.X
    OP = mybir.AluOpType
    AF = mybir.ActivationFunctionType

    nc = bacc.Bacc("TRN2", target_bir_lowering=False, debug=False,
                   num_devices=NCORES)

    xl_ap = nc.dram_tensor("xl", [C, N], f32, kind="ExternalInput").ap()
    wqt_ap = nc.dram_tensor("wqt", [C, C8], f32, kind="ExternalInput").ap()
    wkt_ap = nc.dram_tensor("wkt", [C, C8], f32, kind="ExternalInput").ap()
    wvt_ap = nc.dram_tensor("wvt", [C, C], f32, kind="ExternalInput").ap()
    wft_ap = nc.dram_tensor("wft", [C, C], f32, kind="ExternalInput").ap()
    id_ap = nc.dram_tensor("ident", [P, P], f32, kind="ExternalInput").ap()
    gpa_ap = nc.dram_tensor("gpa", [P, 1], f32, kind="ExternalInput").ap()
    gca_ap = nc.dram_tensor("gca", [P, 1], f32, kind="ExternalInput").ap()
    bnw_ap = nc.dram_tensor("bnw", [P, 4], f32, kind="ExternalInput").ap()
    bnb_ap = nc.dram_tensor("bnb", [P, 4], f32, kind="ExternalInput").ap()
    y_ap = nc.dram_tensor("y", [C, NH], f32, kind="ExternalOutput").ap()

    xl_r = xl_ap.rearrange("(i p) n -> i p n", p=P)      # [4,128,4096]
    wqt_r = wqt_ap.rearrange("(i p) o -> p i o", p=P)    # [128,4,64]
    wkt_r = wkt_ap.rearrange("(i p) o -> p i o", p=P)
    wvt_r = wvt_ap.rearrange("(i p) o -> p i o", p=P)    # [128,4,512]
    wft_r = wft_ap.rearrange("(i p) o -> p i o", p=P)

    NG = NH // NB    # 4 query groups
    MT = N // P      # 32 m-chunks

    with tile.TileContext(nc) as tc:
        with (
            tc.tile_pool(name="consts", bufs=1) as consts,
            tc.tile_pool(name="xr", bufs=1) as xr_pool,
            tc.tile_pool(name="big", bufs=1) as big,
            tc.tile_pool(name="w8k", bufs=3) as w8k,
            tc.tile_pool(name="w8r", bufs=2) as w8r,
            tc.tile_pool(name="small", bufs=3) as small,
            tc.tile_pool(name="stp", bufs=6) as stp,
            tc.tile_pool(name="small2", bufs=2) as small2,
            tc.tile_pool(name="misc", bufs=1) as misc,
            tc.tile_pool(name="pmm", bufs=3, space="PSUM") as pmm,
            tc.tile_pool(name="pacc", bufs=4, space="PSUM") as pacc,
            tc.tile_pool(name="prow", bufs=1, space="PSUM") as prow,
            tc.tile_pool(name="dram", bufs=1, space="DRAM") as dram,
        ):
            # ---------------- Phase A: loads + fp32r rounding ----------------
            ident = consts.tile([P, P], f32)
            nc.sync.dma_start(ident[:], id_ap)
            ident_r = consts.tile([P, P], f32r)
            nc.vector.tensor_copy(ident_r[:], ident[:])
            gpa = consts.tile([P, 1], f32)
            nc.sync.dma_start(gpa[:], gpa_ap)
            gca = consts.tile([P, 1], f32)
            nc.sync.dma_start(gca[:], gca_ap)
            bnw = consts.tile([P, 4], f32)
            nc.sync.dma_start(bnw[:], bnw_ap)
            bnb = consts.tile([P, 4], f32)
            nc.sync.dma_start(bnb[:], bnb_ap)
            ones_bf = consts.tile([P, P], bf16)
            nc.vector.memset(ones_bf[:], 1.0)

            wq_r = consts.tile([P, 4, C8], f32r)
            wk_r = consts.tile([P, 4, C8], f32r)
            wv_r = consts.tile([P, 4, NB], f32r)
            wf_r = consts.tile([P, 4, NB], f32r)

            def load_w(name, dst, srcap, fd):
                stg = w8k.tile([P, 4, fd], f32, tag="w8", name=f"stg_{name}")
                nc.sync.dma_start(stg[:], srcap)
                nc.vector.tensor_copy(dst[:], stg[:])

            X = xr_pool.tile([P, 4, N], f32r)
            load_w("wq", wq_r, wqt_r, C8)
            load_w("wk", wk_r, wkt_r, C8)
            for hb in range(2):
                for i in range(4):
                    stx = w8k.tile([P, NH], f32, tag="w8", name=f"stx{i}_{hb}")
                    nc.sync.dma_start(stx[:], xl_r[i, :, hb * NH:(hb + 1) * NH])
                    nc.vector.tensor_copy(X[:, i, hb * NH:(hb + 1) * NH], stx[:])
            load_w("wv", wv_r, wvt_r, NB)
            load_w("wf", wf_r, wft_r, NB)

            # ---------------- Phase B: q, k (bf16, zero-padded to 128) -------
            k_bf = big.tile([P, N], bf16)
            q_bf = big.tile([P, NH], bf16)
            nc.vector.memset(k_bf[:], 0.0)
            nc.vector.memset(q_bf[:], 0.0)
            def emit_qk(which, dst, wt, nb):
                ps = pmm.tile([P, NB], f32, tag="mm", name=f"qk{which}_{nb}")
                for i in range(4):
                    nc.tensor.matmul(ps[:C8, :], wt[:, i, :],
                                     X[:, i, nb * NB:(nb + 1) * NB],
                                     start=(i == 0), stop=(i == 3))
                nc.vector.tensor_copy(dst[:C8, nb * NB:(nb + 1) * NB],
                                      ps[:C8, :])

            # left-half blocks first: they only need the first 2048 columns
            # of every channel chunk, which land ~12us into the x DMA
            for nb in range(4):
                emit_qk(0, k_bf, wk_r, nb)
            for nb in range(4):
                emit_qk(1, q_bf, wq_r, nb)
            for nb in range(4, 8):
                emit_qk(0, k_bf, wk_r, nb)

            # pack for row-tiled E^T: k rows 64-127 = k shifted left one
            # 128-chunk (so chunk pair (mt, mt+1) computes in one PE pass);
            # q rows 64-127 = copy of q. SBUF->SBUF DMA shifts partitions.
            nc.sync.dma_start(k_bf[C8:P, 0:N - P], k_bf[0:C8, P:N])
            nc.sync.dma_start(q_bf[C8:P, :], q_bf[0:C8, :])

            # ---------------- Phase D1: ce = x x^T (fp32r) -------------------
            ce_acc = [pacc.tile([P, NB], f32, tag="acc", name=f"ce{ct}")
                      for ct in range(4)]

            def emit_xt(nt):
                tp = pmm.tile([P, NB], f32r, tag="mm", name=f"cetp{nt}")
                for i in range(4):
                    nc.tensor.transpose(tp[:, i * P:(i + 1) * P],
                                        X[:, i, nt * P:(nt + 1) * P],
                                        ident_r[:])
                xt = small.tile([P, NB], f32r, tag="xt", name=f"xt{nt}")
                nc.vector.tensor_copy(xt[:], tp[:])
                return xt

            xt_cur = emit_xt(0)
            for nt in range(MT):
                xt_next = emit_xt(nt + 1) if nt + 1 < MT else None
                for ct in range(4):
                    nc.tensor.matmul(ce_acc[ct], xt_cur[:, ct * P:(ct + 1) * P],
                                     xt_cur[:],
                                     start=(nt == 0), stop=(nt == MT - 1))
                xt_cur = xt_next

            # ---------------- Phase D2: channel softmax ----------------------
            # (ACT/DVE only; phase C below keeps PE busy meanwhile)
            ce_sb = w8k.tile([P, 4, NB], f32, tag="w8", name="ce_sb")
            cmin = misc.tile([P, 4], f32)
            csum = misc.tile([P, 4], f32)
            for ct in range(4):
                nc.scalar.copy(ce_sb[:, ct, :], ce_acc[ct])
                nc.vector.tensor_reduce(out=cmin[:, ct:ct + 1],
                                        in_=ce_sb[:, ct, :], axis=AX, op=OP.min)
                nc.scalar.activation(out=ce_sb[:, ct, :], in_=ce_sb[:, ct, :],
                                     func=AF.Exp, bias=cmin[:, ct:ct + 1],
                                     scale=-1.0,
                                     accum_out=csum[:, ct:ct + 1])
            crcp = misc.tile([P, 4], f32)
            nc.vector.reciprocal(crcp[:], csum[:])
            nc.vector.tensor_scalar(out=crcp[:], in0=crcp[:],
                                    scalar1=gca[:, 0:1], scalar2=None,
                                    op0=OP.mult)
            cattn_r = w8r.tile([P, 4, NB], f32r, tag="w8r", name="cattn_r")
            for ct in range(4):
                nc.vector.tensor_scalar(out=cattn_r[:, ct, :],
                                        in0=ce_sb[:, ct, :],
                                        scalar1=crcp[:, ct:ct + 1], scalar2=None,
                                        op0=OP.mult)

            # ---------------- Phase C: vT (PE work overlapping D2) -----------
            vT = big.tile([P, MT, NB], bf16)
            for mt in range(MT):
                ps = pmm.tile([P, NB], f32, tag="mm", name=f"vt{mt}")
                for i in range(4):
                    nc.tensor.matmul(ps[:], X[:, i, mt * P:(mt + 1) * P],
                                     wv_r[:, i, :],
                                     start=(i == 0), stop=(i == 3))
                nc.vector.tensor_copy(vT[:, mt, :], ps[:])

            # ---------------- Phase D3: cattn^T ------------------------------
            catT = big.tile([P, 4, NB], f32r)
            for dt in range(4):
                tp = pmm.tile([P, NB], f32r, tag="mm", name=f"catp{dt}")
                for ct in range(4):
                    nc.tensor.transpose(tp[:, ct * P:(ct + 1) * P],
                                        cattn_r[:, ct, dt * P:(dt + 1) * P],
                                        ident_r[:])
                nc.vector.tensor_copy(catT[:, dt, :], tp[:])

            # ---------------- Phase E: position attention, pipelined ---------
            ysum = misc.tile([P, 4, 4], f32)
            ysq = misc.tile([P, 4, 4], f32)
            ysp = dram.tile([C, NH], f32)

            def emit_fuse(g, s_f):
                gsf = slice(g * NB, (g + 1) * NB)
                yps = [pacc.tile([P, NB], f32, tag="acc", name=f"y{g}_{ot}")
                       for ot in range(4)]
                for ct in range(4):
                    for ot in range(4):
                        nc.tensor.matmul(yps[ot], wf_r[:, ct, ot * P:(ot + 1) * P],
                                         s_f[:, ct, :],
                                         start=(ct == 0), stop=(ct == 3))
                for ot in range(4):
                    yst = small.tile([P, NB], f32, tag="yst", name=f"yst{g}_{ot}")
                    nc.vector.tensor_copy(yst[:], yps[ot])
                    nc.vector.tensor_reduce(out=ysum[:, ot, g:g + 1],
                                            in_=yst[:], axis=AX, op=OP.add)
                    sq = small2.tile([P, NB], f32, tag="sqs", name=f"sq{g}_{ot}")
                    nc.scalar.activation(out=sq[:], in_=yst[:], func=AF.Square,
                                         bias=0.0, scale=1.0,
                                         accum_out=ysq[:, ot, g:g + 1])
                    nc.sync.dma_start(ysp[ot * P:(ot + 1) * P, gsf], yst[:])

            def emit_ca(g):
                gsc = slice(g * NB, (g + 1) * NB)
                ca_g = w8k.tile([P, 4, NB], f32, tag="w8", name=f"ca{g}")
                for ct in range(4):
                    cp = pmm.tile([P, NB], f32, tag="mm", name=f"cap{g}_{ct}")
                    for dt in range(4):
                        nc.tensor.matmul(cp[:], catT[:, dt, ct * P:(ct + 1) * P],
                                         X[:, dt, gsc],
                                         start=(dt == 0), stop=(dt == 3))
                    nc.vector.tensor_copy(ca_g[:, ct, :], cp[:])
                return ca_g

            prev = None  # (g, s_f) awaiting fuse
            ca_cur = emit_ca(0)
            for g in range(NG):
                gs = slice(g * NB, (g + 1) * NB)
                pv = [pacc.tile([P, NB], f32, tag="acc", name=f"pv{g}_{ct}")
                      for ct in range(4)]
                pr_ = prow.tile([P, NB], f32, tag="rs", name=f"rs{g}")

                def emit_st_pair(mt, g=g, gs=gs):
                    # two K=64 matmuls run concurrently in PE row groups
                    # 0-63 (chunk mt) and 64-127 (chunk mt+1, via shifted k)
                    p1 = pmm.tile([P, NB], f32, tag="mm", name=f"e{g}_{mt}")
                    p2 = pmm.tile([P, NB], f32, tag="mm", name=f"e{g}_{mt + 1}")
                    nc.tensor.matmul(p1[:], k_bf[0:C8, mt * P:(mt + 1) * P],
                                     q_bf[0:C8, gs], start=True, stop=True,
                                     tile_position=(0, 0))
                    nc.tensor.matmul(p2[:], k_bf[C8:P, mt * P:(mt + 1) * P],
                                     q_bf[C8:P, gs], start=True, stop=True,
                                     tile_position=(64, 0))
                    st1 = stp.tile([P, NB], bf16, tag="st", name=f"st{g}_{mt}")
                    nc.scalar.activation(out=st1[:], in_=p1[:], func=AF.Exp)
                    st2 = stp.tile([P, NB], bf16, tag="st", name=f"st{g}_{mt + 1}")
                    nc.scalar.activation(out=st2[:], in_=p2[:], func=AF.Exp)
                    return [st1, st2]

                sts = emit_st_pair(0)
                for mt in range(MT):
                    if mt % 2 == 0 and mt + 2 < MT:
                        sts.extend(emit_st_pair(mt + 2))
                    st_cur = sts[mt]
                    for ct in range(4):
                        nc.tensor.matmul(pv[ct], vT[:, mt, ct * P:(ct + 1) * P],
                                         st_cur[:],
                                         start=(mt == 0), stop=(mt == MT - 1))
                    nc.tensor.matmul(pr_, ones_bf[:], st_cur[:],
                                     start=(mt == 0), stop=(mt == MT - 1))

                # ACT part of the epilogue first: frees the PV psum slots for
                # the next group as soon as possible
                rr = small2.tile([P, NB], f32, tag="rsb", name=f"rr{g}")
                nc.vector.reciprocal(rr[:], pr_)
                s0 = w8k.tile([P, 4, NB], f32, tag="w8", name=f"s0_{g}")
                for ct in range(4):
                    nc.scalar.mul(s0[:, ct, :], pv[ct], gpa[:, 0:1])

                # previous group's fuse + next group's ca: PE work (and DVE
                # evacs ahead of the epilogue chain in the DVE queue)
                if prev is not None:
                    emit_fuse(*prev)
                ca_g = ca_cur
                if g + 1 < NG:
                    ca_cur = emit_ca(g + 1)

                # DVE chain: s = s0/rowsum + ca + 2x
                s_f = w8r.tile([P, 4, NB], f32r, tag="w8r", name=f"sf{g}")
                for ct in range(4):
                    nc.vector.tensor_tensor(out=s0[:, ct, :], in0=s0[:, ct, :],
                                            in1=rr[:], op=OP.mult)
                    nc.vector.tensor_add(s0[:, ct, :], s0[:, ct, :],
                                         ca_g[:, ct, :])
                    xs = X[:, ct, gs].bitcast(f32)
                    nc.vector.tensor_add(s0[:, ct, :], s0[:, ct, :], xs)
                    nc.vector.tensor_tensor(out=s_f[:, ct, :],
                                            in0=s0[:, ct, :], in1=xs, op=OP.add)
                prev = (g, s_f)

            # ---------------- Phase F: BN via two AllReduces -----------------
            # AR1 covers groups 0-2 and runs hidden under group-3 compute
            # (also absorbs accumulated cross-core skew); AR2 covers only
            # group 3 at the tail.
            stats1 = misc.tile([P, 2, 4, 1], f32)
            nc.vector.tensor_reduce(out=stats1[:, 0, :, :],
                                    in_=ysum[:, :, 0:3], axis=AX, op=OP.add)
            nc.vector.tensor_reduce(out=stats1[:, 1, :, :],
                                    in_=ysq[:, :, 0:3], axis=AX, op=OP.add)
            sin1 = dram.tile([P, 8], f32)
            sout1 = dram.tile([P, 8], f32)
            nc.sync.dma_start(sin1[:], stats1.rearrange("p a b c -> p (a b c)"))
            nc.gpsimd.collective_compute(
                "AllReduce", OP.add,
                replica_groups=[list(range(NCORES))],
                ins=[sin1.opt()], outs=[sout1.opt()],
            )

            emit_fuse(*prev)

            stats2 = misc.tile([P, 2, 4, 1], f32)
            nc.vector.tensor_copy(stats2[:, 0, :, :], ysum[:, :, 3:4])
            nc.vector.tensor_copy(stats2[:, 1, :, :], ysq[:, :, 3:4])
            sin = dram.tile([P, 8], f32)
            sout = dram.tile([P, 8], f32)
            nc.sync.dma_start(sin[:], stats2.rearrange("p a b c -> p (a b c)"))
            nc.gpsimd.collective_compute(
                "AllReduce", OP.add,
                replica_groups=[list(range(NCORES))],
                ins=[sin.opt()], outs=[sout.opt()],
            )

            # prefetch the spilled y tiles while the collective runs
            ylds = []
            for ot in range(4):
                yt = w8k.tile([P, NH], f32, tag="w8", name=f"ld{ot}")
                nc.sync.dma_start(yt[:], ysp[ot * P:(ot + 1) * P, :])
                ylds.append((ot, yt))

            gst = misc.tile([P, 8], f32)
            nc.sync.dma_start(gst[:], sout[:])
            gst1 = misc.tile([P, 8], f32)
            nc.sync.dma_start(gst1[:], sout1[:])
            nc.vector.tensor_tensor(out=gst[:], in0=gst[:], in1=gst1[:],
                                    op=OP.add)
            mean = misc.tile([P, 4], f32)
            msq = misc.tile([P, 4], f32)
            nc.vector.tensor_scalar(out=mean[:], in0=gst[:, 0:4],
                                    scalar1=1.0 / CNT, scalar2=None, op0=OP.mult)
            nc.vector.tensor_scalar(out=msq[:], in0=gst[:, 4:8],
                                    scalar1=1.0 / CNT, scalar2=None, op0=OP.mult)
            var = misc.tile([P, 4], f32)
            nc.vector.tensor_tensor(out=var[:], in0=mean[:], in1=mean[:],
                                    op=OP.mult)
            nc.vector.tensor_tensor(out=var[:], in0=msq[:], in1=var[:],
                                    op=OP.subtract)
            eps_t = misc.tile([P, 1], f32)
            nc.vector.memset(eps_t[:], BN_EPS)
            sd = misc.tile([P, 4], f32)
            nc.scalar.activation(out=sd[:], in_=var[:], func=AF.Sqrt,
                                 bias=eps_t[:, 0:1], scale=1.0)
            rstd = misc.tile([P, 4], f32)
            nc.vector.reciprocal(rstd[:], sd[:])
            a_t = misc.tile([P, 4], f32)
            nc.vector.tensor_tensor(out=a_t[:], in0=bnw[:], in1=rstd[:],
                                    op=OP.mult)
            b_t = misc.tile([P, 4], f32)
            nc.vector.tensor_tensor(out=b_t[:], in0=mean[:], in1=a_t[:],
                                    op=OP.mult)
            nc.vector.tensor_tensor(out=b_t[:], in0=bnb[:], in1=b_t[:],
                                    op=OP.subtract)

            for ot, yt in ylds:
                nc.scalar.activation(out=yt[:], in_=yt[:], func=AF.Relu,
                                     scale=a_t[:, ot:ot + 1],
                                     bias=b_t[:, ot:ot + 1])
                nc.sync.dma_start(y_ap[ot * P:(ot + 1) * P, :], yt[:])

    nc.compile()
    return nc


def kernel(x, wq, wk, wv, w_fuse, gamma_pa, gamma_ca, bn_scale, bn_bias):
    from concourse.bass_utils import run_bass_kernel_spmd

    if "nc" not in _CACHE:
        _CACHE["nc"] = _build_program()
    nc = _CACHE["nc"]

    x = np.asarray(x, dtype=np.float32)
    xf = x.reshape(B, C, N)
    wqt = np.ascontiguousarray(np.asarray(wq, np.float32).T)   # [512, 64]
    wkt = np.ascontiguousarray(np.asarray(wk, np.float32).T)
    wvt = np.ascontiguousarray(np.asarray(wv, np.float32).T)   # [512, 512]
    wft = np.ascontiguousarray(np.asarray(w_fuse, np.float32).T)
    ident = np.eye(P, dtype=np.float32)
    gpa = np.full((P, 1), np.float32(np.asarray(gamma_pa).reshape(-1)[0]),
                  np.float32)
    gca = np.full((P, 1), np.float32(np.asarray(gamma_ca).reshape(-1)[0]),
                  np.float32)
    bnw = np.ascontiguousarray(np.asarray(bn_scale, np.float32).reshape(4, P).T)
    bnb = np.ascontiguousarray(np.asarray(bn_bias, np.float32).reshape(4, P).T)

    in_maps = []
    for c in range(NCORES):
        b, h = divmod(c, 2)
        own = xf[b][:, h * NH:(h + 1) * NH]
        other = xf[b][:, (1 - h) * NH:(2 - h) * NH]
        xl = np.ascontiguousarray(np.concatenate([own, other], axis=1))
        in_maps.append({
            "xl": xl, "wqt": wqt, "wkt": wkt, "wvt": wvt, "wft": wft,
            "ident": ident, "gpa": gpa, "gca": gca, "bnw": bnw, "bnb": bnb,
        })

    res = run_bass_kernel_spmd(nc, in_maps, core_ids=list(range(NCORES)))
    out = np.empty((B, C, N), dtype=np.float32)
    for c in range(NCORES):
        b, h = divmod(c, 2)
        out[b][:, h * NH:(h + 1) * NH] = res.results[c]["y"]
    return out.reshape(B, C, H, W)
